# revision 1
# baseline (speedup 1.0000x reference)
"""nn_AdapFilter3d Trainium2 kernel — 8-core SPMD (data-parallel over (B,C)).

out[b,c,z,y,x] = sum_{i,j,k} pad(input)[b,c,z+i-1,y+j-1,x+k-1] * F[b,c,z,y,x,i,j,k]

Strategy (per NeuronCore, 4 of the 32 independent (b,c) slices):
  - Partition layout p = 64*s_local + y for a pair of slices; free dims (z, x)
    with zero halos. The y-shift of each tap is baked into three separately
    DMA'd copies of the (small) input, since DVE lanes cannot cross partitions.
  - F is pre-transposed on the host to [..., y, tap, x] so each tap's x-row is
    contiguous in SBUF (full-rate VectorE reads) while DMA runs stay 6912 B.
  - Per (pair, z-chunk): 27 VectorE multiplies (shifted-window x F tap window),
    each forwarded into PSUM by a TensorE identity matmul with accumulation
    (does all 26 adds on the otherwise idle PE); ScalarE evicts PSUM; DMA out.
  - Large DMAs alternate between the two HWDGE rings (sync/scalar) so the two
    64-partition halves stream concurrently through all 16 SDMA engines.

Self-contained: hardcodes shapes from the problem spec; only needs the
concourse/axon environment on sys.path.
"""

import numpy as np

import concourse.bacc as bacc
import concourse.tile as tile
from concourse import mybir
from concourse.bass_utils import run_bass_kernel_spmd

B, C, D, H, W = 2, 16, 32, 64, 64
TAPS = 27
N_CORES = 8
S = (B * C) // N_CORES  # 4 slices per core
PAIRS = S // 2
ZC = 8  # z planes per chunk
NCHUNK = D // ZC
XT = W * TAPS  # 1728
PLANES = D + 2
COLS = W + 2
FD = ZC * W  # 512

F32 = mybir.dt.float32
PROD_DT = mybir.dt.bfloat16  # product dtype (PSUM accumulation stays fp32)


def _build(prod_dt=PROD_DT):
    nc = bacc.Bacc()
    x_ext = nc.declare_dram_parameter("input", [S, D, H, W], F32, isOutput=False)
    f_ext = nc.declare_dram_parameter("F", [S, D, H, XT], F32, isOutput=False)
    id_ext = nc.declare_dram_parameter("ident", [128, 128], prod_dt, isOutput=False)
    z_ext = nc.declare_dram_parameter("zeros", [1, PLANES * COLS], F32, isOutput=False)
    out_ext = nc.declare_dram_parameter("out", [S, D, H, W], F32, isOutput=True)

    with tile.TileContext(nc) as tc:
        with (
            tc.tile_pool(name="const", bufs=1) as cpool,
            tc.tile_pool(name="xp", bufs=6) as xpool,
            tc.tile_pool(name="fp", bufs=2) as fpool,
            tc.tile_pool(name="prod", bufs=3) as ppool,
            tc.tile_pool(name="osb", bufs=2) as opool,
            tc.tile_pool(name="ps", bufs=2, space="PSUM") as pspool,
        ):
            ident = cpool.tile([128, 128], prod_dt)
            nc.sync.dma_start(ident[:], id_ext[:])

            for pair in range(PAIRS):
                xps = []
                for j in range(3):
                    dy = j - 1
                    xp = xpool.tile([128, PLANES * COLS], F32, tag="xp")
                    xp3 = xp[:].rearrange("p (zp c) -> p zp c", zp=PLANES)
                    nc.vector.memset(xp3[:, 0, :], 0.0)
                    nc.vector.memset(xp3[:, PLANES - 1, :], 0.0)
                    nc.vector.memset(xp3[:, 1 : PLANES - 1, 0], 0.0)
                    nc.vector.memset(xp3[:, 1 : PLANES - 1, COLS - 1], 0.0)
                    y_lo = max(0, -dy)
                    y_hi = min(H, H - dy)
                    halo_y = H - 1 if dy > 0 else 0
                    for s in range(2):
                        dma_eng = nc.sync if s == 0 else nc.scalar
                        if dy != 0:
                            dma_eng.dma_start(
                                xp[64 * s + halo_y : 64 * s + halo_y + 1, :],
                                z_ext[:, :],
                            )
                        sl = pair * 2 + s
                        dma_eng.dma_start(
                            xp3[
                                64 * s + y_lo : 64 * s + y_hi,
                                1 : PLANES - 1,
                                1 : COLS - 1,
                            ],
                            x_ext[sl, :, y_lo + dy : y_hi + dy, :].rearrange(
                                "z y x -> y z x"
                            ),
                        )
                    xps.append(xp3)

                for zc in range(NCHUNK):
                    ft = fpool.tile([128, ZC * XT], F32, tag="ft")
                    ft3 = ft[:].rearrange("p (z q) -> p z q", z=ZC)
                    ft4 = ft[:].rearrange("p (z t x) -> p z t x", z=ZC, t=TAPS)
                    for s in range(2):
                        sl = pair * 2 + s
                        dma_eng = nc.sync if s == 0 else nc.scalar
                        dma_eng.dma_start(
                            ft3[64 * s : 64 * s + 64, :, :],
                            f_ext[sl, zc * ZC : (zc + 1) * ZC, :, :].rearrange(
                                "z y q -> y z q"
                            ),
                        )
                    psumt = pspool.tile([128, FD], F32, tag="ps")
                    for t in range(TAPS):
                        i, j, k = t // 9, (t // 3) % 3, t % 3
                        prod = ppool.tile([128, FD], prod_dt, tag="prod")
                        prod3 = prod[:].rearrange("p (z x) -> p z x", z=ZC)
                        nc.vector.tensor_mul(
                            prod3[:, :, :],
                            xps[j][:, zc * ZC + i : zc * ZC + i + ZC, k : k + W],
                            ft4[:, :, t, :],
                        )
                        nc.tensor.matmul(
                            psumt[:],
                            ident[:],
                            prod[:],
                            start=(t == 0),
                            stop=(t == TAPS - 1),
                        )
                    osb = opool.tile([128, FD], F32, tag="osb")
                    nc.scalar.copy(osb[:], psumt[:])
                    osb3 = osb[:].rearrange("p (z x) -> p z x", z=ZC)
                    for s in range(2):
                        sl = pair * 2 + s
                        dma_eng = nc.scalar if s == 0 else nc.sync
                        dma_eng.dma_start(
                            out_ext[sl, zc * ZC : (zc + 1) * ZC, :, :].rearrange(
                                "z y x -> y z x"
                            ),
                            osb3[64 * s : 64 * s + 64, :, :],
                        )
    nc.compile()
    return nc


_NC_CACHE = {}


def kernel(input: np.ndarray, F: np.ndarray) -> np.ndarray:
    input = np.asarray(input)
    F = np.asarray(F)
    assert input.shape == (B, C, D, H, W), input.shape
    assert F.shape == (B, C, D, H, W, 3, 3, 3), F.shape

    if "nc" not in _NC_CACHE:
        _NC_CACHE["nc"] = _build()
    nc = _NC_CACHE["nc"]

    xs = np.ascontiguousarray(input.reshape(B * C, D, H, W).astype(np.float32))
    # host pre-transpose: [..., y, x, tap] -> [..., y, tap, x]
    fs = np.ascontiguousarray(
        F.reshape(B * C, D, H, W, TAPS)
        .transpose(0, 1, 2, 4, 3)
        .reshape(B * C, D, H, XT)
        .astype(np.float32)
    )
    ident = np.eye(128, dtype=mybir.dt.np(PROD_DT))
    zeros = np.zeros((1, PLANES * COLS), dtype=np.float32)
    in_maps = [
        {
            "input": xs[c * S : (c + 1) * S],
            "F": fs[c * S : (c + 1) * S],
            "ident": ident,
            "zeros": zeros,
        }
        for c in range(N_CORES)
    ]
    res = run_bass_kernel_spmd(nc, in_maps, core_ids=list(range(N_CORES)))
    out = np.concatenate([res.results[c]["out"] for c in range(N_CORES)], axis=0)
    return np.ascontiguousarray(out.reshape(B, C, D, H, W).astype(np.float32))
